# revision 4
# baseline (speedup 1.0000x reference)
"""MoE FFN (top-2 routing) Trainium2 kernel.

Strategy (8 NeuronCores, SPMD via run_bass_kernel_spmd):
  Pass 1 (router, data-parallel over tokens): each core takes N/8 = 512
    tokens (x pre-transposed to [D, 512] by the host), computes logits =
    x @ gate_w in fp32 on the PE, derives top-2 expert mask and softmax
    weights entirely on-device, and emits:
      - wfull [512, E]: per-token router weight for every expert
        (nonzero exactly at the token's top-2 experts)
      - aux partials [1, 17]: softmax-prob column sums (8), top-1
        one-hot column sums (8), sum of logits^2 (1)
  Host dispatch ("all-to-all"): tokens are gathered per expert from the
    nonzero pattern of wfull, laid out transposed ([D, C], zero-padded
    to capacity C), and shipped to the expert's core. Pure data
    movement - no arithmetic on the host.
  Pass 2 (expert FFN, expert-parallel): core e holds expert e's w1/w2
    resident in SBUF and streams its gathered tokens through
      hT = gelu(w1.T @ xT + b1)   [F on partitions]
      y  = wtok * (hT.T @ w2 + b2) [tokens on partitions]
    with float32r matmuls (fp32 data, 1 cycle/row on the PE for free
    dim >= 256). Also combines the pass-1 aux partials into the scalar
    aux loss on-device.
  Host combine: scatter-add of the (already router-weighted) per-expert
    outputs back to [B, T, D]. Each token receives exactly its two
    expert contributions.
"""

import os
import numpy as np

import concourse.bass as bass
import concourse.mybir as mybir
import concourse.tile as tile
from concourse import bacc
from concourse.alu_op_type import AluOpType
from concourse.bass_utils import run_bass_kernel_spmd

f32 = mybir.dt.float32
f32r = mybir.dt.float32r
AX = mybir.AxisListType
ACT = mybir.ActivationFunctionType

B, T, D, E, F = 2, 2048, 1024, 8, 2048
N = B * T           # 4096 tokens
NCORES = 8
TPC = N // NCORES   # 512 tokens per core in pass 1
DCH = D // 128      # 8 d-chunks
FCH = F // 128      # 16 f-chunks
TBLK = 384          # pass-2 token block (moving dim; >=256 keeps f32r fast)

Z_LOSS_COEF = 1e-3
TOP_K = 2

_nc_cache = {}


def _build_pass1():
    """Router kernel: one core's 512-token shard."""
    nc = bacc.Bacc("TRN2", target_bir_lowering=False, debug=False,
                   num_devices=NCORES)
    xt_d = nc.dram_tensor("xt", [D, TPC], f32, kind="ExternalInput").ap()
    gwt_d = nc.dram_tensor("gwt", [128, DCH * E], f32, kind="ExternalInput").ap()
    wfull_d = nc.dram_tensor("wfull", [TPC, E], f32, kind="ExternalOutput").ap()
    auxp_d = nc.dram_tensor("auxp", [1, 17], f32, kind="ExternalOutput").ap()

    with tile.TileContext(nc) as tc:
        with tc.tile_pool(name="sb", bufs=1) as sb, \
             tc.tile_pool(name="sc", bufs=2) as sc, \
             tc.tile_pool(name="ps", bufs=2, space="PSUM") as ps:
            xt = sb.tile([128, DCH, TPC], f32)
            nc.sync.dma_start(out=xt[:], in_=xt_d.rearrange("(c p) t -> p c t", p=128))
            gw = sb.tile([128, DCH, E], f32)
            nc.sync.dma_start(out=gw[:], in_=gwt_d.rearrange("p (c e) -> p c e", e=E))

            acc = sb.tile([128, 17], f32)   # [probs 8 | onehot 8 | z 1]
            nc.vector.memset(acc[:], 0.0)
            ones = sb.tile([128, 1], f32)
            nc.vector.memset(ones[:], 1.0)

            nblk = TPC // 128
            for b in range(nblk):
                lg = ps.tile([128, E], f32, tag="lg")
                for d in range(DCH):
                    # logits[tok, e] += xt[d-chunk, tok].T @ gw[d-chunk, e]
                    nc.tensor.matmul(lg[:], lhsT=xt[:, d, bass.ts(b, 128)],
                                     rhs=gw[:, d, :],
                                     start=(d == 0), stop=(d == DCH - 1))
                t1 = sc.tile([128, 1], f32, tag="t1")
                nc.vector.tensor_reduce(t1[:], lg[:], axis=AX.X, op=AluOpType.max)
                t1n = sc.tile([128, 1], f32, tag="t1n")
                nc.vector.tensor_scalar_mul(t1n[:], t1[:], -1.0)
                eq1 = sc.tile([128, E], f32, tag="eq1")
                nc.vector.tensor_scalar(out=eq1[:], in0=lg[:], scalar1=t1[:],
                                        scalar2=None, op0=AluOpType.is_equal)
                msk = sc.tile([128, E], f32, tag="msk")
                nc.vector.scalar_tensor_tensor(out=msk[:], in0=eq1[:], scalar=-1e30,
                                               in1=lg[:], op0=AluOpType.mult,
                                               op1=AluOpType.add)
                t2 = sc.tile([128, 1], f32, tag="t2")
                nc.vector.tensor_reduce(t2[:], msk[:], axis=AX.X, op=AluOpType.max)
                eq2 = sc.tile([128, E], f32, tag="eq2")
                nc.vector.tensor_scalar(out=eq2[:], in0=msk[:], scalar1=t2[:],
                                        scalar2=None, op0=AluOpType.is_equal)
                # top-2 softmax: wa = 1/(1+exp(t2-t1)) (top-1), wb = 1-wa
                d21 = sc.tile([128, 1], f32, tag="d21")
                nc.vector.tensor_sub(d21[:], t2[:], t1[:])
                ex = sc.tile([128, 1], f32, tag="ex")
                nc.scalar.activation(ex[:], d21[:], ACT.Exp)
                den = sc.tile([128, 1], f32, tag="den")
                nc.vector.tensor_scalar_add(den[:], ex[:], 1.0)
                wa = sc.tile([128, 1], f32, tag="wa")
                nc.vector.reciprocal(wa[:], den[:])
                wb = sc.tile([128, 1], f32, tag="wb")
                nc.vector.tensor_mul(wb[:], ex[:], wa[:])
                # wfull = wa*eq1 + wb*eq2
                wtmp = sc.tile([128, E], f32, tag="wtmp")
                nc.vector.tensor_scalar_mul(wtmp[:], eq1[:], wa[:])
                wf = sc.tile([128, E], f32, tag="wf")
                nc.vector.scalar_tensor_tensor(out=wf[:], in0=eq2[:], scalar=wb[:],
                                               in1=wtmp[:], op0=AluOpType.mult,
                                               op1=AluOpType.add)
                nc.sync.dma_start(out=wfull_d[bass.ts(b, 128), :], in_=wf[:])
                # aux partials
                pr = sc.tile([128, E], f32, tag="pr")
                sume = sc.tile([128, 1], f32, tag="sume")
                nc.scalar.activation(pr[:], lg[:], ACT.Exp, bias=t1n[:],
                                     accum_out=sume[:])
                rs = sc.tile([128, 1], f32, tag="rs")
                nc.vector.reciprocal(rs[:], sume[:])
                nc.vector.scalar_tensor_tensor(out=acc[:, 0:E], in0=pr[:],
                                               scalar=rs[:], in1=acc[:, 0:E],
                                               op0=AluOpType.mult,
                                               op1=AluOpType.add)
                nc.vector.tensor_add(acc[:, E:2 * E], acc[:, E:2 * E], eq1[:])
                sq = sc.tile([128, E], f32, tag="sq")
                zs = sc.tile([128, 1], f32, tag="zs")
                nc.scalar.activation(sq[:], lg[:], ACT.Square, accum_out=zs[:])
                nc.vector.tensor_add(acc[:, 16:17], acc[:, 16:17], zs[:])
            # column sums over the 128 partitions via ones-matmul
            accp = ps.tile([1, 17], f32, tag="accp")
            nc.tensor.matmul(accp[:], lhsT=ones[:], rhs=acc[:], start=True, stop=True)
            accs = sc.tile([1, 17], f32, tag="accs")
            nc.vector.tensor_copy(accs[:], accp[:])
            nc.sync.dma_start(out=auxp_d[:], in_=accs[:])

    nc.compile()
    return nc


def _build_pass2(C):
    """Expert FFN kernel: one expert's C gathered tokens (C % TBLK == 0)."""
    nc = bacc.Bacc("TRN2", target_bir_lowering=False, debug=False,
                   num_devices=NCORES)
    xgt_d = nc.dram_tensor("xgt", [D, C], f32r, kind="ExternalInput").ap()
    w1_d = nc.dram_tensor("w1", [D, F], f32r, kind="ExternalInput").ap()
    w2_d = nc.dram_tensor("w2", [F, D], f32r, kind="ExternalInput").ap()
    b1t_d = nc.dram_tensor("b1t", [128, FCH], f32, kind="ExternalInput").ap()
    b2b_d = nc.dram_tensor("b2b", [128, D], f32, kind="ExternalInput").ap()
    wtok_d = nc.dram_tensor("wtok", [128, C // 128], f32, kind="ExternalInput").ap()
    auxp_d = nc.dram_tensor("auxp", [NCORES, 17], f32, kind="ExternalInput").ap()
    y_d = nc.dram_tensor("y", [C, D], f32, kind="ExternalOutput").ap()
    aux_d = nc.dram_tensor("aux", [1, 1], f32, kind="ExternalOutput").ap()

    nblk = C // TBLK
    nsub = TBLK // 128

    with tile.TileContext(nc) as tc:
        with tc.tile_pool(name="wt", bufs=1) as wt, \
             tc.tile_pool(name="xg", bufs=2) as xg, \
             tc.tile_pool(name="hp", bufs=1) as hp, \
             tc.tile_pool(name="yo", bufs=3) as yo, \
             tc.tile_pool(name="sm", bufs=1) as sm, \
             tc.tile_pool(name="ph", bufs=3, space="PSUM") as ph, \
             tc.tile_pool(name="pa", bufs=1, space="PSUM") as pa, \
             tc.tile_pool(name="py", bufs=2, space="PSUM") as py:
            # resident weights
            w1 = wt.tile([128, DCH, F], f32r)
            nc.sync.dma_start(out=w1[:], in_=w1_d.rearrange("(c p) f -> p c f", p=128))
            w2 = wt.tile([128, FCH, D], f32r)
            nc.sync.dma_start(out=w2[:], in_=w2_d.rearrange("(c p) d -> p c d", p=128))
            b1t = wt.tile([128, FCH], f32)
            nc.sync.dma_start(out=b1t[:], in_=b1t_d[:])
            b2b = wt.tile([128, D], f32)
            nc.sync.dma_start(out=b2b[:], in_=b2b_d[:])
            wtok = wt.tile([128, C // 128], f32)
            nc.sync.dma_start(out=wtok[:], in_=wtok_d[:])

            # aux combine (tiny, once)
            auxp = sm.tile([NCORES, 17], f32)
            nc.sync.dma_start(out=auxp[:], in_=auxp_d[:])
            ones8 = sm.tile([NCORES, 1], f32)
            nc.vector.memset(ones8[:], 1.0)
            auxs = sm.tile([1, 17], f32)
            auxt = pa.tile([1, 17], f32, tag="auxt")
            nc.tensor.matmul(auxt[:], lhsT=ones8[:], rhs=auxp[:], start=True,
                             stop=True)
            nc.vector.tensor_copy(auxs[:], auxt[:])
            prod = sm.tile([1, E], f32)
            nc.vector.tensor_mul(prod[:], auxs[:, 0:E], auxs[:, E:2 * E])
            psum_ = sm.tile([1, 1], f32)
            nc.vector.tensor_reduce(psum_[:], prod[:], axis=AX.X, op=AluOpType.add)
            zt = sm.tile([1, 1], f32)
            nc.vector.tensor_scalar_mul(zt[:], auxs[:, 16:17],
                                        float(Z_LOSS_COEF / (N * E)))
            auxo = sm.tile([1, 1], f32)
            nc.vector.scalar_tensor_tensor(out=auxo[:], in0=psum_[:],
                                           scalar=float(E) / (float(N) * float(N)),
                                           in1=zt[:], op0=AluOpType.mult,
                                           op1=AluOpType.add)
            nc.sync.dma_start(out=aux_d[:], in_=auxo[:])

            # main FFN loop
            for t in range(nblk):
                xt = xg.tile([128, DCH, TBLK], f32r, tag="xt")
                nc.sync.dma_start(
                    out=xt[:],
                    in_=xgt_d[:, bass.ts(t, TBLK)].rearrange("(c p) t -> p c t", p=128))
                hT = hp.tile([128, FCH, TBLK], f32r, tag="hT")
                for f in range(FCH):
                    hps = ph.tile([128, TBLK], f32, tag="hps")
                    for d in range(DCH):
                        nc.tensor.matmul(
                            hps[:],
                            lhsT=w1[:, d, bass.ts(f, 128)],
                            rhs=xt[:, d, :],
                            start=(d == 0), stop=(d == DCH - 1))
                    # hT = gelu(w1.T x + b1), erf flavor
                    nc.scalar.activation(hT[:, f, :], hps[:], ACT.Gelu,
                                         bias=b1t[:, f:f + 1])
                for s in range(nsub):
                    yt = yo.tile([128, D], f32, tag="yt")
                    for n in range(2):
                        yps = py.tile([128, 512], f32, tag="yps")
                        for f in range(FCH):
                            nc.tensor.matmul(
                                yps[:],
                                lhsT=hT[:, f, bass.ts(s, 128)],
                                rhs=w2[:, f, bass.ts(n, 512)],
                                start=(f == 0), stop=(f == FCH - 1))
                        # y = wtok * (psum + b2)
                        tb = yo.tile([128, 512], f32, tag="tb")
                        nc.vector.tensor_add(tb[:], yps[:], b2b[:, bass.ts(n, 512)])
                        nc.scalar.activation(yt[:, bass.ts(n, 512)], tb[:], ACT.Copy,
                                             scale=wtok[:, t * nsub + s:t * nsub + s + 1])
                    nc.sync.dma_start(out=y_d[bass.ts(t * nsub + s, 128), :],
                                      in_=yt[:])

    nc.compile()
    return nc


def _get_pass1():
    if "p1" not in _nc_cache:
        _nc_cache["p1"] = _build_pass1()
    return _nc_cache["p1"]


def _get_pass2(C):
    key = ("p2", C)
    if key not in _nc_cache:
        _nc_cache[key] = _build_pass2(C)
    return _nc_cache[key]


def run(inputs, trace=False, trace_cores=None):
    x = np.ascontiguousarray(np.asarray(inputs["x"], dtype=np.float32))
    gate_w = np.ascontiguousarray(np.asarray(inputs["gate_w"], dtype=np.float32))
    ew1 = np.ascontiguousarray(np.asarray(inputs["expert_w1"], dtype=np.float32))
    eb1 = np.ascontiguousarray(np.asarray(inputs["expert_b1"], dtype=np.float32))
    ew2 = np.ascontiguousarray(np.asarray(inputs["expert_w2"], dtype=np.float32))
    eb2 = np.ascontiguousarray(np.asarray(inputs["expert_b2"], dtype=np.float32))

    xf = x.reshape(N, D)
    xT = np.ascontiguousarray(xf.T)                       # [D, N]
    gwt = np.ascontiguousarray(
        gate_w.reshape(DCH, 128, E).transpose(1, 0, 2).reshape(128, DCH * E))

    perf = {}
    kw = dict(trace=trace)
    if trace and trace_cores is not None:
        kw["trace_cores"] = trace_cores

    # ---- pass 1: router ----
    nc1 = _get_pass1()
    in1 = [{"xt": np.ascontiguousarray(xT[:, c * TPC:(c + 1) * TPC]), "gwt": gwt}
           for c in range(NCORES)]
    r1 = run_bass_kernel_spmd(nc1, in1, core_ids=list(range(NCORES)), **kw)
    perf["pass1_ns"] = r1.exec_time_ns
    wfull = np.concatenate([r1.results[c]["wfull"] for c in range(NCORES)], axis=0)
    auxp = np.concatenate([r1.results[c]["auxp"] for c in range(NCORES)], axis=0)

    # ---- host dispatch (data movement only) ----
    idx = [np.nonzero(wfull[:, e])[0] for e in range(E)]
    maxc = max(len(i) for i in idx)
    C = max(TBLK, ((maxc + TBLK - 1) // TBLK) * TBLK)
    in2 = []
    for e in range(E):
        ie = idx[e]
        xg = np.zeros((D, C), np.float32)
        xg[:, :len(ie)] = xT[:, ie]
        wt = np.zeros(C, np.float32)
        wt[:len(ie)] = wfull[ie, e]
        in2.append({
            "xgt": xg,
            "w1": ew1[e],
            "w2": ew2[e],
            "b1t": np.ascontiguousarray(eb1[e].reshape(FCH, 128).T),
            "b2b": np.ascontiguousarray(np.broadcast_to(eb2[e], (128, D))),
            "wtok": np.ascontiguousarray(wt.reshape(C // 128, 128).T),
            "auxp": auxp,
        })

    # ---- pass 2: expert FFN ----
    nc2 = _get_pass2(C)
    r2 = run_bass_kernel_spmd(nc2, in2, core_ids=list(range(NCORES)), **kw)
    perf["pass2_ns"] = r2.exec_time_ns
    perf["C"] = C

    # ---- host combine (scatter-add of the two expert contributions) ----
    out = np.zeros((N, D), np.float32)
    for e in range(E):
        ie = idx[e]
        out[ie] += r2.results[e]["y"][:len(ie)]
    aux = np.float32(r2.results[0]["aux"][0, 0])
    return out.reshape(B, T, D), aux, perf


def kernel(**inputs):
    out, aux, _ = run(inputs, trace=bool(int(os.environ.get("KERNEL_TRACE", "0"))))
    return out, aux


# revision 5
# speedup vs baseline: 1.2168x; 1.2168x over previous
"""MoE FFN (top-2 routing) Trainium2 kernel.

Strategy (8 NeuronCores, SPMD via run_bass_kernel_spmd):
  Pass 1 (router, data-parallel over tokens): each core takes N/8 = 512
    tokens (x pre-transposed to [D, 512] by the host), computes logits =
    x @ gate_w in fp32 on the PE (gate stationary, tokens moving, then a
    PE transpose back to token-partition layout), derives the top-2
    expert mask and softmax weights on-device with batched DVE ops, and
    emits:
      - wfull [512, E]: per-token router weight for every expert
        (nonzero exactly at the token's top-2 experts)
      - aux partials [1, 17]: softmax-prob column sums (8), top-1
        one-hot column sums (8), sum of logits^2 (1)
  Host dispatch ("all-to-all"): tokens are gathered per expert from the
    nonzero pattern of wfull, laid out transposed ([D, C], zero-padded
    to capacity C), and shipped to the expert's core. Pure data
    movement - no arithmetic on the host.
  Pass 2 (expert FFN, expert-parallel): core e holds expert e's w1/w2
    resident in SBUF and streams its gathered tokens through
      hT = gelu(w1.T @ xT + b1)   [F on partitions]
      y  = wtok * (hT.T @ w2 + b2) [tokens on partitions]
    with float32r matmuls (fp32 data rounded to 11-bit mantissa, 1
    cycle/row on the PE for free dim >= 256). Weights are split into
    per-chunk tiles and DMA-ordered so the PE starts as soon as the
    first chunks land. Also combines the pass-1 aux partials into the
    scalar aux loss on-device.
  Host combine: scatter-add of the (already router-weighted) per-expert
    outputs back to [B, T, D]. Each token receives exactly its two
    expert contributions.
"""

import os
import numpy as np

import concourse.bass as bass
import concourse.mybir as mybir
import concourse.tile as tile
from concourse import bacc
from concourse.alu_op_type import AluOpType
from concourse.bass_utils import run_bass_kernel_spmd
from concourse.masks import make_identity

f32 = mybir.dt.float32
f32r = mybir.dt.float32r
AX = mybir.AxisListType
ACT = mybir.ActivationFunctionType
TT = AluOpType

B, T, D, E, F = 2, 2048, 1024, 8, 2048
N = B * T           # 4096 tokens
NCORES = 8
TPC = N // NCORES   # 512 tokens per core in pass 1
DCH = D // 128      # 8 d-chunks
FCH = F // 128      # 16 f-chunks
TBLK = 384          # pass-2 token block (moving dim; >=256 keeps f32r fast)

Z_LOSS_COEF = 1e-3

_nc_cache = {}


def _build_pass1():
    """Router kernel: one core's 512-token shard."""
    nc = bacc.Bacc("TRN2", target_bir_lowering=False, debug=False,
                   num_devices=NCORES)
    xt_d = nc.dram_tensor("xt", [D, TPC], f32, kind="ExternalInput").ap()
    gwt_d = nc.dram_tensor("gwt", [128, DCH * E], f32, kind="ExternalInput").ap()
    wfull_d = nc.dram_tensor("wfull", [TPC, E], f32, kind="ExternalOutput").ap()
    auxp_d = nc.dram_tensor("auxp", [1, 17], f32, kind="ExternalOutput").ap()

    NB = TPC // 128  # 4 token blocks per core

    with tile.TileContext(nc) as tc:
        with tc.tile_pool(name="sb", bufs=1) as sb, \
             tc.tile_pool(name="ps", bufs=2, space="PSUM") as ps, \
             tc.tile_pool(name="pl", bufs=1, space="PSUM") as pl:
            gw = sb.tile([128, DCH, E], f32)
            nc.sync.dma_start(out=gw[:], in_=gwt_d.rearrange("p (c e) -> p c e", e=E))
            xts = []
            for d in range(DCH):
                xtd = sb.tile([128, TPC], f32, name=f"xt{d}", tag=f"xt{d}")
                nc.sync.dma_start(out=xtd[:], in_=xt_d[bass.ts(d, 128), :])
                xts.append(xtd)
            ident = sb.tile([128, 128], f32)
            make_identity(nc, ident[:])
            ones = sb.tile([128, 1], f32)
            nc.vector.memset(ones[:], 1.0)

            # logits^T [E, TPC] on PSUM (gate stationary, tokens moving, fp32)
            lgT = pl.tile([E, TPC], f32, tag="lgT")
            for d in range(DCH):
                nc.tensor.matmul(lgT[:], lhsT=gw[:, d, :], rhs=xts[d][:],
                                 start=(d == 0), stop=(d == DCH - 1))
            lgTs = sb.tile([E, TPC], f32)
            nc.scalar.copy(lgTs[:], lgT[:])
            # transpose back to [128 tokens, E] per block, gather into SBUF
            lgs = sb.tile([128, NB, E], f32)
            for b in range(NB):
                lgp = ps.tile([128, E], f32, tag="lgp")
                nc.tensor.transpose(lgp[:], lgTs[:, bass.ts(b, 128)],
                                    ident[:E, :E])
                nc.scalar.copy(lgs[:, b, :], lgp[:])

            def bc(ap):  # [128, NB] -> [128, NB, E] stride-0 broadcast
                return ap[:, :, None].broadcast_to([128, NB, E])

            acc = sb.tile([128, 17], f32)   # [probs 8 | onehot 8 | z 1]
            t1 = sb.tile([128, NB], f32)
            nc.vector.tensor_reduce(t1[:], lgs[:], axis=AX.X, op=TT.max)
            eq1 = sb.tile([128, NB, E], f32)
            nc.vector.tensor_tensor(eq1[:], lgs[:], bc(t1), TT.is_equal)
            msk = sb.tile([128, NB, E], f32)
            nc.vector.scalar_tensor_tensor(out=msk[:], in0=eq1[:], scalar=-1e30,
                                           in1=lgs[:], op0=TT.mult, op1=TT.add)
            t2 = sb.tile([128, NB], f32)
            nc.vector.tensor_reduce(t2[:], msk[:], axis=AX.X, op=TT.max)
            eq2 = sb.tile([128, NB, E], f32)
            nc.vector.tensor_tensor(eq2[:], msk[:], bc(t2), TT.is_equal)
            # top-2 softmax weights: wa = 1/(1+exp(t2-t1)), wb = 1-wa
            d21 = sb.tile([128, NB], f32)
            nc.vector.tensor_sub(d21[:], t2[:], t1[:])
            ex = sb.tile([128, NB], f32)
            nc.scalar.activation(ex[:], d21[:], ACT.Exp)
            den = sb.tile([128, NB], f32)
            nc.vector.tensor_scalar_add(den[:], ex[:], 1.0)
            wa = sb.tile([128, NB], f32)
            nc.vector.reciprocal(wa[:], den[:])
            wb = sb.tile([128, NB], f32)
            nc.vector.tensor_mul(wb[:], ex[:], wa[:])
            # wfull = wa*eq1 + wb*eq2
            wf = sb.tile([128, NB, E], f32)
            nc.vector.tensor_tensor(wf[:], eq1[:], bc(wa), TT.mult)
            wf2 = sb.tile([128, NB, E], f32)
            nc.vector.tensor_tensor(wf2[:], eq2[:], bc(wb), TT.mult)
            nc.vector.tensor_add(wf[:], wf[:], wf2[:])
            nc.sync.dma_start(out=wfull_d.rearrange("(b p) e -> p b e", p=128),
                              in_=wf[:])
            # softmax probs for aux
            sub = sb.tile([128, NB, E], f32)
            nc.vector.tensor_tensor(sub[:], lgs[:], bc(t1), TT.subtract)
            pr = sb.tile([128, NB, E], f32)
            nc.scalar.activation(pr[:], sub[:], ACT.Exp)
            se = sb.tile([128, NB], f32)
            nc.vector.tensor_reduce(se[:], pr[:], axis=AX.X, op=TT.add)
            rs = sb.tile([128, NB], f32)
            nc.vector.reciprocal(rs[:], se[:])
            prn = sb.tile([128, NB, E], f32)
            nc.vector.tensor_tensor(prn[:], pr[:], bc(rs), TT.mult)
            # block-sums into acc
            nc.vector.tensor_add(acc[:, 0:E], prn[:, 0, :], prn[:, 1, :])
            nc.vector.tensor_add(acc[:, 0:E], acc[:, 0:E], prn[:, 2, :])
            nc.vector.tensor_add(acc[:, 0:E], acc[:, 0:E], prn[:, 3, :])
            nc.vector.tensor_add(acc[:, E:2 * E], eq1[:, 0, :], eq1[:, 1, :])
            nc.vector.tensor_add(acc[:, E:2 * E], acc[:, E:2 * E], eq1[:, 2, :])
            nc.vector.tensor_add(acc[:, E:2 * E], acc[:, E:2 * E], eq1[:, 3, :])
            sq = sb.tile([128, NB, E], f32)
            nc.scalar.activation(sq[:], lgs[:], ACT.Square,
                                 accum_out=acc[:, 16:17])
            # column sums over the 128 partitions via ones-matmul
            accp = ps.tile([1, 17], f32, tag="accp")
            nc.tensor.matmul(accp[:], lhsT=ones[:], rhs=acc[:], start=True,
                             stop=True)
            accs = sb.tile([1, 17], f32)
            nc.vector.tensor_copy(accs[:], accp[:])
            nc.sync.dma_start(out=auxp_d[:], in_=accs[:])

    nc.compile()
    return nc


def _build_pass2(C):
    """Expert FFN kernel: one expert's C gathered tokens (C % TBLK == 0)."""
    nc = bacc.Bacc("TRN2", target_bir_lowering=False, debug=False,
                   num_devices=NCORES)
    xgt_d = nc.dram_tensor("xgt", [D, C], f32r, kind="ExternalInput").ap()
    w1_d = nc.dram_tensor("w1", [D, F], f32r, kind="ExternalInput").ap()
    w2_d = nc.dram_tensor("w2", [F, D], f32r, kind="ExternalInput").ap()
    b1t_d = nc.dram_tensor("b1t", [128, FCH], f32, kind="ExternalInput").ap()
    b2b_d = nc.dram_tensor("b2b", [128, D], f32, kind="ExternalInput").ap()
    wtok_d = nc.dram_tensor("wtok", [128, C // 128], f32, kind="ExternalInput").ap()
    auxp_d = nc.dram_tensor("auxp", [NCORES, 17], f32, kind="ExternalInput").ap()
    y_d = nc.dram_tensor("y", [C, D], f32, kind="ExternalOutput").ap()
    aux_d = nc.dram_tensor("aux", [1, 1], f32, kind="ExternalOutput").ap()

    nblk = C // TBLK
    nsub = TBLK // 128

    with tile.TileContext(nc) as tc:
        with tc.tile_pool(name="wt", bufs=1) as wt, \
             tc.tile_pool(name="xg", bufs=2) as xg, \
             tc.tile_pool(name="hp", bufs=1) as hp, \
             tc.tile_pool(name="yo", bufs=2) as yo, \
             tc.tile_pool(name="sm", bufs=1) as sm, \
             tc.tile_pool(name="ph", bufs=3, space="PSUM") as ph, \
             tc.tile_pool(name="pa", bufs=1, space="PSUM") as pa, \
             tc.tile_pool(name="py", bufs=2, space="PSUM") as py:

            def xgt_block(t):
                xts = []
                for d in range(DCH):
                    xtd = xg.tile([128, TBLK], f32r, name=f"xt{t}_{d}",
                                  tag=f"xtd{d}")
                    nc.sync.dma_start(
                        out=xtd[:],
                        in_=xgt_d[bass.ts(d, 128), bass.ts(t, TBLK)])
                    xts.append(xtd)
                return xts

            # DMA issue order: first block's tokens + w1 first (mm1 needs
            # them), then small constants, then w2 (mm2 starts ~25us in).
            xts0 = xgt_block(0)
            w1c = []
            for d in range(DCH):
                w1d = wt.tile([128, F], f32r, name=f"w1_{d}")
                nc.sync.dma_start(out=w1d[:], in_=w1_d[bass.ts(d, 128), :])
                w1c.append(w1d)
            b1t = wt.tile([128, FCH], f32)
            nc.sync.dma_start(out=b1t[:], in_=b1t_d[:])
            wtok = wt.tile([128, C // 128], f32)
            nc.sync.dma_start(out=wtok[:], in_=wtok_d[:])
            w2c = []
            for f in range(FCH):
                w2f = wt.tile([128, D], f32r, name=f"w2_{f}")
                nc.sync.dma_start(out=w2f[:], in_=w2_d[bass.ts(f, 128), :])
                w2c.append(w2f)
            b2b = wt.tile([128, D], f32)
            nc.sync.dma_start(out=b2b[:], in_=b2b_d[:])

            # aux combine (tiny, once)
            auxp = sm.tile([NCORES, 17], f32)
            nc.sync.dma_start(out=auxp[:], in_=auxp_d[:])
            ones8 = sm.tile([NCORES, 1], f32)
            nc.vector.memset(ones8[:], 1.0)
            auxs = sm.tile([1, 17], f32)
            auxt = pa.tile([1, 17], f32, tag="auxt")
            nc.tensor.matmul(auxt[:], lhsT=ones8[:], rhs=auxp[:], start=True,
                             stop=True)
            nc.vector.tensor_copy(auxs[:], auxt[:])
            prod = sm.tile([1, E], f32)
            nc.vector.tensor_mul(prod[:], auxs[:, 0:E], auxs[:, E:2 * E])
            psum_ = sm.tile([1, 1], f32)
            nc.vector.tensor_reduce(psum_[:], prod[:], axis=AX.X, op=TT.add)
            zt = sm.tile([1, 1], f32)
            nc.vector.tensor_scalar_mul(zt[:], auxs[:, 16:17],
                                        float(Z_LOSS_COEF / (N * E)))
            auxo = sm.tile([1, 1], f32)
            nc.vector.scalar_tensor_tensor(out=auxo[:], in0=psum_[:],
                                           scalar=float(E) / (float(N) * float(N)),
                                           in1=zt[:], op0=TT.mult, op1=TT.add)
            nc.sync.dma_start(out=aux_d[:], in_=auxo[:])

            # main FFN loop
            for t in range(nblk):
                xts = xts0 if t == 0 else xgt_block(t)
                hT = hp.tile([128, FCH, TBLK], f32r, tag="hT")
                for f in range(FCH):
                    hps = ph.tile([128, TBLK], f32, tag="hps")
                    for d in range(DCH):
                        nc.tensor.matmul(hps[:], lhsT=w1c[d][:, bass.ts(f, 128)],
                                         rhs=xts[d][:],
                                         start=(d == 0), stop=(d == DCH - 1))
                    # hT = gelu(w1.T x + b1), erf flavor
                    nc.scalar.activation(hT[:, f, :], hps[:], ACT.Gelu,
                                         bias=b1t[:, f:f + 1])
                for s in range(nsub):
                    yt = yo.tile([128, D], f32, tag="yt")
                    for n in range(2):
                        yps = py.tile([128, 512], f32, tag="yps")
                        for f in range(FCH):
                            nc.tensor.matmul(yps[:],
                                             lhsT=hT[:, f, bass.ts(s, 128)],
                                             rhs=w2c[f][:, bass.ts(n, 512)],
                                             start=(f == 0), stop=(f == FCH - 1))
                        # y = wtok * (psum + b2)
                        tb = yo.tile([128, 512], f32, tag="tb")
                        nc.vector.tensor_add(tb[:], yps[:], b2b[:, bass.ts(n, 512)])
                        nc.scalar.activation(
                            yt[:, bass.ts(n, 512)], tb[:], ACT.Copy,
                            scale=wtok[:, t * nsub + s:t * nsub + s + 1])
                    nc.sync.dma_start(out=y_d[bass.ts(t * nsub + s, 128), :],
                                      in_=yt[:])

    nc.compile()
    return nc


def _get_pass1():
    if "p1" not in _nc_cache:
        _nc_cache["p1"] = _build_pass1()
    return _nc_cache["p1"]


def _get_pass2(C):
    key = ("p2", C)
    if key not in _nc_cache:
        _nc_cache[key] = _build_pass2(C)
    return _nc_cache[key]


def run(inputs, trace=False, trace_cores=None):
    x = np.ascontiguousarray(np.asarray(inputs["x"], dtype=np.float32))
    gate_w = np.ascontiguousarray(np.asarray(inputs["gate_w"], dtype=np.float32))
    ew1 = np.ascontiguousarray(np.asarray(inputs["expert_w1"], dtype=np.float32))
    eb1 = np.ascontiguousarray(np.asarray(inputs["expert_b1"], dtype=np.float32))
    ew2 = np.ascontiguousarray(np.asarray(inputs["expert_w2"], dtype=np.float32))
    eb2 = np.ascontiguousarray(np.asarray(inputs["expert_b2"], dtype=np.float32))

    xf = x.reshape(N, D)
    xT = np.ascontiguousarray(xf.T)                       # [D, N]
    gwt = np.ascontiguousarray(
        gate_w.reshape(DCH, 128, E).transpose(1, 0, 2).reshape(128, DCH * E))

    perf = {}
    kw = dict(trace=trace)
    if trace and trace_cores is not None:
        kw["trace_cores"] = trace_cores

    # ---- pass 1: router ----
    nc1 = _get_pass1()
    in1 = [{"xt": np.ascontiguousarray(xT[:, c * TPC:(c + 1) * TPC]), "gwt": gwt}
           for c in range(NCORES)]
    r1 = run_bass_kernel_spmd(nc1, in1, core_ids=list(range(NCORES)), **kw)
    perf["pass1_ns"] = r1.exec_time_ns
    wfull = np.concatenate([r1.results[c]["wfull"] for c in range(NCORES)], axis=0)
    auxp = np.concatenate([r1.results[c]["auxp"] for c in range(NCORES)], axis=0)

    # ---- host dispatch (data movement only) ----
    idx = [np.nonzero(wfull[:, e])[0] for e in range(E)]
    maxc = max(len(i) for i in idx)
    C = max(TBLK, ((maxc + TBLK - 1) // TBLK) * TBLK)
    in2 = []
    for e in range(E):
        ie = idx[e]
        xg = np.zeros((D, C), np.float32)
        xg[:, :len(ie)] = xT[:, ie]
        wt = np.zeros(C, np.float32)
        wt[:len(ie)] = wfull[ie, e]
        in2.append({
            "xgt": xg,
            "w1": ew1[e],
            "w2": ew2[e],
            "b1t": np.ascontiguousarray(eb1[e].reshape(FCH, 128).T),
            "b2b": np.ascontiguousarray(np.broadcast_to(eb2[e], (128, D))),
            "wtok": np.ascontiguousarray(wt.reshape(C // 128, 128).T),
            "auxp": auxp,
        })

    # ---- pass 2: expert FFN ----
    nc2 = _get_pass2(C)
    r2 = run_bass_kernel_spmd(nc2, in2, core_ids=list(range(NCORES)), **kw)
    perf["pass2_ns"] = r2.exec_time_ns
    perf["C"] = C
    perf["r1"] = r1
    perf["r2"] = r2

    # ---- host combine (scatter-add of the two expert contributions) ----
    out = np.zeros((N, D), np.float32)
    for e in range(E):
        ie = idx[e]
        out[ie] += r2.results[e]["y"][:len(ie)]
    aux = np.float32(r2.results[0]["aux"][0, 0])
    return out.reshape(B, T, D), aux, perf


def kernel(**inputs):
    out, aux, _ = run(inputs, trace=bool(int(os.environ.get("KERNEL_TRACE", "0"))))
    return out, aux
